# revision 9
# baseline (speedup 1.0000x reference)
"""CrossAttention kernel for 8 trn2 NeuronCores — optimized v2.

Sharding: core c handles batch b = c//4 and head group hg = c%4 (4 of 16 heads).
Key optimizations over the baseline:
  * Host-side key compaction: masked-out keys (bernoulli 0.5 mask) contribute
    exactly zero after softmax, so only attended keys are shipped/projected/
    attended (padded to a 128 multiple, bias handles the pad tail). ~2x less
    attention work.
  * bf16 everywhere on the device data path (f32 PSUM accumulation).
  * x and ctx pipelines interleaved; PSUM->SBUF copies balanced across
    DVE and Act (GPSIMD cannot touch PSUM on TRN2); sim/exp/AV
    software-pipelined; next chunk's first head prefetched under the
    out-projection so the Act exp stream never drains.
  * bf16 ReduceScatter per 1024-query chunk overlapped with the next
    chunk's attention; final LayerNorms pushed to the scheduler's queue
    tail (tile_wait_until) so RS-gated ops never head-of-line block.
"""

import sys

sys.path.insert(0, "/opt/trn_rl_repo")

import numpy as np

import concourse.bass as bass
import concourse.mybir as mybir
import concourse.tile as tile
from concourse.bass_utils import run_bass_kernel_spmd
from concourse.masks import make_identity

# problem constants (hardcoded per the harness contract)
B, N, M, DIM = 2, 2048, 2048, 1024
HEADS, DH = 16, 64
INNER = HEADS * DH
H_PER = 4  # heads per core
HS = H_PER * DH  # 256 inner columns per core
NT = N // 128  # 16 seq tiles
KT = DIM // 128  # 8 contraction tiles
QW = 1024  # query chunk width (2 chunks)
EPS = 1e-5
SCALE = DH ** -0.5
NEG_BIG = -1.0e30

F32 = mybir.dt.float32
BF16 = mybir.dt.bfloat16

_cache = {}


def split_multi_waits(nc):
    """This container's walrus supports a single sync-wait per instruction.
    Move extra waits onto same-engine NOPs placed immediately before."""
    for f in nc.m.functions:
        for blk in f.blocks:
            insts = list(blk.instructions)
            if not any(
                i.sync_info is not None and len(i.sync_info.on_wait) > 1
                for i in insts
            ):
                continue
            new_list = []
            for inst in insts:
                si = inst.sync_info
                if si is not None and len(si.on_wait) > 1:
                    waits = list(si.on_wait)
                    for k, w in enumerate(waits[:-1]):
                        new_list.append(
                            mybir.InstNoOp(
                                name=f"{inst.name}_ws{k}",
                                sync_info=mybir.SyncInfo(on_wait=[w], on_update=[]),
                                bass_nofuse=True,
                                engine=inst.engine,
                            )
                        )
                    inst.sync_info = mybir.SyncInfo(
                        on_wait=[waits[-1]], on_update=list(si.on_update)
                    )
                new_list.append(inst)
            blk.instructions = new_list


def build_program(kpt, rlast=128):
    """kpt = number of 128-row key tiles after host-side mask compaction;
    rlast = rows in the last (partial) key tile, 1..128."""
    KP = kpt * 128
    KR = (kpt - 1) * 128 + rlast  # real (unpadded-to-tile) key count bound
    # the null key/value ride in the last tile's first pad row when it has
    # room; otherwise (full last tile) they get their own 1-row iteration
    null_merged = rlast < 128

    def krows(kt):
        if kt < kpt - 1:
            return 128
        return rlast + 1 if null_merged else rlast
    nc = bass.Bass("TRN2", target_bir_lowering=False, debug=False, num_devices=8)
    AF = mybir.ActivationFunctionType

    x = nc.dram_tensor("x", [N, DIM], BF16, kind="ExternalInput")
    ctx_in = nc.dram_tensor("ctx", [KP, DIM], BF16, kind="ExternalInput")
    maskbias = nc.dram_tensor("maskbias", [128, kpt + 1], F32, kind="ExternalInput")
    nk_in = nc.dram_tensor("nk", [DH, 1], BF16, kind="ExternalInput")
    nvr_in = nc.dram_tensor("nvr", [1, DH + 1], BF16, kind="ExternalInput")
    wq_in = nc.dram_tensor("wq", [128, KT * HS], BF16, kind="ExternalInput")
    wk_in = nc.dram_tensor("wk", [128, KT * HS], BF16, kind="ExternalInput")
    wv_in = nc.dram_tensor("wv", [128, KT * HS], BF16, kind="ExternalInput")
    wout_in = nc.dram_tensor("wout", [DH, H_PER * DIM], BF16, kind="ExternalInput")
    gout_in = nc.dram_tensor("gout", [DIM], F32, kind="ExternalInput")
    y = nc.dram_tensor("y", [N // 4, DIM], F32, kind="ExternalOutput")

    with tile.TileContext(nc) as tc:
        with tc.tile_pool(name="persist", bufs=1) as persist, \
             tc.tile_pool(name="dram", bufs=1, space="DRAM") as dram:
            ident = persist.tile([128, 128], BF16)
            make_identity(nc, ident[:])
            # rstd*SCALE = exp(-0.5*ln(DH*var + DH*eps))
            eps64_t = persist.tile([128, 1], F32, tag="eps64")
            nc.vector.memset(eps64_t[:], EPS * DH)
            eps_t = persist.tile([128, 1], F32, tag="eps")
            nc.vector.memset(eps_t[:], EPS)
            mb = persist.tile([128, kpt + 1], F32, tag="mb")
            nvr = persist.tile([1, DH + 1], BF16, tag="nvr")
            ones64 = persist.tile([1, DH], BF16, tag="ones64")
            nc.vector.memset(ones64[:], 1.0)
            gout_b = persist.tile([128, DIM], F32, tag="gout_b")

            qT = persist.tile([DH, H_PER, N], BF16, tag="qT")
            kT = persist.tile([DH, H_PER, KP + 1], BF16, tag="kT")
            vhat = persist.tile([128, H_PER, kpt, DH + 1], BF16, tag="vhat")
            wout = persist.tile([DH, H_PER, DIM], BF16, tag="wout")
            nullcol = KR if null_merged else KP
            # ones column of vhat (written before v-proj fills cols 0..DH-1)
            nc.vector.memset(vhat[:, :, :, DH : DH + 1], 1.0)

            # ---------------- Phase AB: x & ctx -> LN/transpose -> q,k,v ----
            with tc.tile_pool(name="stage", bufs=8) as stage, \
                 tc.tile_pool(name="xs_p", bufs=6) as xs_p, \
                 tc.tile_pool(name="stats", bufs=8) as stats_p, \
                 tc.tile_pool(name="xnT_p", bufs=1) as xnT_p, \
                 tc.tile_pool(name="ctxT_p", bufs=1) as ctxT_p, \
                 tc.tile_pool(name="w_p", bufs=1) as w_p, \
                 tc.tile_pool(name="ps_tp", bufs=3, space="PSUM") as ps_tp, \
                 tc.tile_pool(name="ps_pr", bufs=3, space="PSUM") as ps_pr, \
                 tc.tile_pool(name="ps_v", bufs=2, space="PSUM") as ps_v:
                xnT = xnT_p.tile([128, KT, N], BF16)
                ctxT = ctxT_p.tile([128, KT, KP], BF16)
                wq = w_p.tile([128, KT, HS], BF16, tag="wq")
                wk = w_p.tile([128, KT, HS], BF16, tag="wk")
                wv = w_p.tile([128, KT, HS], BF16, tag="wv")
                # wv first: the first PE matmul (v-proj of ctx tile 0) needs it
                nc.scalar.dma_start(wv[:].rearrange("p t n -> p (t n)"), wv_in[:])
                nc.scalar.dma_start(mb[:], maskbias[:])
                nc.scalar.dma_start(nvr[:], nvr_in[:])
                nc.scalar.dma_start(
                    kT[:, :, nullcol : nullcol + 1],
                    nk_in[:].unsqueeze(1).broadcast_to([DH, H_PER, 1]),
                )
                if null_merged:
                    # null value row: partition `rlast` of the last key tile
                    nc.scalar.dma_start(
                        vhat[rlast : rlast + 1, :, kpt - 1, :],
                        nvr_in[:].unsqueeze(1).broadcast_to([1, H_PER, DH + 1]),
                    )

                def emit_ctx_tile(t):
                    r = 128 if t < kpt - 1 else rlast
                    c_t = stage.tile([128, DIM], BF16, tag="c_t")
                    qeng = nc.sync if t == 0 else nc.scalar
                    qeng.dma_start(
                        c_t[0:r, :], ctx_in[t * 128 : t * 128 + r, :]
                    )
                    ctp = ps_tp.tile([128, KT, 128], BF16, tag="tp")
                    for d in range(KT):
                        nc.tensor.transpose(
                            ctp[:, d, 0:r], c_t[0:r, d * 128 : (d + 1) * 128],
                            ident[0:r, 0:r],
                        )
                    if t % 3 == 2:
                        nc.scalar.copy(
                            ctxT[:, :, t * 128 : t * 128 + r], ctp[:, :, 0:r]
                        )
                    else:
                        nc.vector.tensor_copy(
                            ctxT[:, :, t * 128 : t * 128 + r], ctp[:, :, 0:r]
                        )
                    # v projection for this key tile
                    psv = ps_v.tile([128, HS], F32, tag="psv")
                    for k in range(KT):
                        nc.tensor.matmul(
                            psv[0:r, :],
                            ctxT[:, k, t * 128 : t * 128 + r],
                            wv[:, k, :],
                            start=(k == 0), stop=(k == KT - 1),
                        )
                    nc.vector.tensor_copy(
                        vhat[0:r, :, t, 0:DH],
                        psv[0:r, :].rearrange("p (h d) -> p h d", d=DH),
                    )

                def emit_x_tile(t):
                    x_t = stage.tile([128, DIM], BF16, tag="x_t")
                    nc.sync.dma_start(x_t[:], x[t * 128 : (t + 1) * 128, :])
                    stats = stats_p.tile([128, 2, 6], F32, tag="stats")
                    xr = x_t[:].rearrange("p (s d) -> p s d", d=512)
                    for s in range(2):
                        nc.vector.bn_stats(stats[:, s, :], xr[:, s, :])
                    mv = stats_p.tile([128, 2], F32, tag="mv")
                    nc.vector.bn_aggr(mv[:], stats[:])
                    lnv = stats_p.tile([128, 1], F32, tag="lnv")
                    nc.scalar.activation(
                        lnv[:], mv[:, 1:2], AF.Ln, bias=eps64_t[:], scale=float(DH)
                    )
                    cs_t = stats_p.tile([128, 1], F32, tag="cs_t")
                    nc.scalar.activation(cs_t[:], lnv[:], AF.Exp, scale=-0.5)
                    xs_t = xs_p.tile([128, DIM], BF16, tag="xs_t")
                    if t < 8:
                        # dense-AB phase: Act has slack, DVE is the binding
                        # engine -> apply LN on Act via Identity(cs*x + nmc)
                        nmc = stats_p.tile([128, 1], F32, tag="nmc")
                        nc.vector.scalar_tensor_tensor(
                            out=nmc[:], in0=mv[:, 0:1], scalar=-1.0, in1=cs_t[:],
                            op0=mybir.AluOpType.mult, op1=mybir.AluOpType.mult,
                        )
                        nc.scalar.activation(
                            xs_t[:], x_t[:], AF.Identity, bias=nmc[:], scale=cs_t[:]
                        )
                    else:
                        # these tiles overlap chunk-0 attention where the Act
                        # exp stream is the bottleneck -> apply LN on DVE
                        nc.vector.tensor_scalar(
                            out=xs_t[:], in0=x_t[:],
                            scalar1=mv[:, 0:1], scalar2=cs_t[:],
                            op0=mybir.AluOpType.subtract,
                            op1=mybir.AluOpType.mult,
                        )
                    xtp = ps_tp.tile([128, KT, 128], BF16, tag="tp")
                    for d in range(KT):
                        nc.tensor.transpose(
                            xtp[:, d, :], xs_t[:, d * 128 : (d + 1) * 128], ident[:]
                        )
                    if t % 3 == 2:
                        nc.scalar.copy(xnT[:, :, t * 128 : (t + 1) * 128], xtp[:])
                    else:
                        nc.vector.tensor_copy(
                            xnT[:, :, t * 128 : (t + 1) * 128], xtp[:]
                        )

                def emit_q_chunk(qc):
                    for p in range(2):
                        psq = ps_pr.tile([128, 512], F32, tag="psq")
                        for k in range(KT):
                            nc.tensor.matmul(
                                psq[:],
                                wq[:, k, p * 128 : (p + 1) * 128],
                                xnT[:, k, qc * 512 : (qc + 1) * 512],
                                start=(k == 0), stop=(k == KT - 1),
                            )
                        nc.scalar.copy(
                            qT[:, 2 * p, qc * 512 : (qc + 1) * 512], psq[0:DH, :]
                        )
                        nc.scalar.copy(
                            qT[:, 2 * p + 1, qc * 512 : (qc + 1) * 512], psq[DH:128, :]
                        )

                def emit_k_chunk(off, w):
                    for p in range(2):
                        psk = ps_pr.tile([128, 512], F32, tag="psq")
                        for k in range(KT):
                            nc.tensor.matmul(
                                psk[:, 0:w],
                                wk[:, k, p * 128 : (p + 1) * 128],
                                ctxT[:, k, off : off + w],
                                start=(k == 0), stop=(k == KT - 1),
                            )
                        nc.vector.tensor_copy(
                            kT[:, 2 * p, off : off + w], psk[0:DH, 0:w]
                        )
                        nc.vector.tensor_copy(
                            kT[:, 2 * p + 1, off : off + w], psk[DH:128, 0:w]
                        )

                k_done = 0
                for t in range(NT):
                    if t < kpt:
                        emit_ctx_tile(t)
                    emit_x_tile(t)
                    if t == 1:
                        nc.scalar.dma_start(
                            wq[:].rearrange("p t n -> p (t n)"), wq_in[:]
                        )
                        nc.scalar.dma_start(
                            wk[:].rearrange("p t n -> p (t n)"), wk_in[:]
                        )
                    if t % 4 == 3:
                        emit_q_chunk(t // 4)
                        # k chunks that are fully transposed by now
                        avail = min((t + 1) * 128, KR)
                        while avail - k_done >= 512:
                            emit_k_chunk(k_done, 512)
                            k_done += 512
                for t in range(NT, kpt):  # kpt > 16 never happens, safety
                    emit_ctx_tile(t)
                if KR - k_done > 0:
                    emit_k_chunk(k_done, KR - k_done)

            # ---------------- Phase C+D: attention, out-proj, RS, LN --------
            nc.scalar.dma_start(wout[:].rearrange("p h n -> p (h n)"), wout_in[:])
            nc.sync.dma_start(
                gout_b[:], gout_in[:].unsqueeze(0).broadcast_to([128, DIM])
            )
            partial = dram.tile([N, DIM], BF16, tag="partial")
            rs_out = dram.tile([N // 4, DIM], BF16, tag="rs_out")
            with tc.tile_pool(name="pt_p", bufs=22) as pt_p, \
                 tc.tile_pool(name="outT_p", bufs=2) as outT_p, \
                 tc.tile_pool(name="nrm", bufs=2) as nrm, \
                 tc.tile_pool(name="part_p", bufs=3) as part_p, \
                 tc.tile_pool(name="ln_p", bufs=2) as ln_p, \
                 tc.tile_pool(name="ps_sim", bufs=2, space="PSUM") as ps_sim, \
                 tc.tile_pool(name="ps_out", bufs=1, space="PSUM") as ps_out, \
                 tc.tile_pool(name="ps_d", bufs=2, space="PSUM") as ps_d:

                def emit_final_ln(j):
                    ln_t = ln_p.tile([128, DIM], BF16, tag="ln_t")
                    nc.sync.dma_start(
                        ln_t[:], rs_out[j * 128 : (j + 1) * 128, :]
                    )
                    stats = ln_p.tile([128, 2, 6], F32, tag="statsd")
                    lr = ln_t[:].rearrange("p (s d) -> p s d", d=512)
                    for s in range(2):
                        nc.vector.bn_stats(stats[:, s, :], lr[:, s, :])
                    mv = ln_p.tile([128, 2], F32, tag="mvd")
                    nc.vector.bn_aggr(mv[:], stats[:])
                    lnv = ln_p.tile([128, 1], F32, tag="lnvd")
                    nc.scalar.activation(lnv[:], mv[:, 1:2], AF.Ln, bias=eps_t[:])
                    rstd = ln_p.tile([128, 1], F32, tag="rstdd")
                    nc.scalar.activation(rstd[:], lnv[:], AF.Exp, scale=-0.5)
                    y_t = ln_p.tile([128, DIM], F32, tag="y_t")
                    nc.vector.tensor_scalar(
                        out=y_t[:], in0=ln_t[:],
                        scalar1=mv[:, 0:1], scalar2=rstd[:],
                        op0=mybir.AluOpType.subtract,
                        op1=mybir.AluOpType.mult,
                    )
                    yo = ln_p.tile([128, DIM], F32, tag="yo")
                    nc.vector.tensor_tensor(
                        out=yo[:], in0=y_t[:], in1=gout_b[:],
                        op=mybir.AluOpType.mult,
                    )
                    nc.scalar.dma_start(y[j * 128 : (j + 1) * 128, :], yo[:])

                pre = []
                for qc in range(N // QW):
                    outT = outT_p.tile([DH, H_PER, QW], BF16, tag="outT")

                    def emit_norm(h_, pso_):
                        # normalize: divide rows 0..DH-1 by the sum row DH
                        rec = nrm.tile([1, QW], BF16, tag="rec")
                        with nc.allow_low_precision(reason="softmax recip"):
                            nc.vector.reciprocal(rec[:], pso_[DH : DH + 1, :])
                        bc = ps_sim.tile([DH, QW], F32, tag="sim")
                        for j in range(QW // 512):
                            nc.tensor.matmul(
                                bc[:, j * 512 : (j + 1) * 512],
                                ones64[:],
                                rec[:, j * 512 : (j + 1) * 512],
                                start=True, stop=True,
                            )
                        o_s = nrm.tile([DH, QW], BF16, tag="o_s")
                        nc.vector.tensor_copy(o_s[:], pso_[0:DH, :])
                        nc.vector.tensor_tensor(
                            out=outT[:, h_, :], in0=o_s[:], in1=bc[:],
                            op=mybir.AluOpType.mult,
                        )

                    def emit_av2(pso_, h_, qc_, pt, kt):
                        vrow = (
                            vhat[0 : krows(kt), h_, kt, :]
                            if kt < kpt
                            else nvr[:]
                        )
                        last = kpt - 1 if null_merged else kpt
                        for j in range(QW // 512):
                            nc.tensor.matmul(
                                pso_[:, j * 512 : (j + 1) * 512],
                                vrow,
                                pt[:, j * 512 : (j + 1) * 512],
                                start=(kt == 0), stop=(kt == last),
                            )

                    def emit_head_sims(qc_, h_, pso_, pending_flush=None,
                                       defer_avs=False):
                        """sims + exps for one head; AVs returned for the
                        caller to place (enables cross-chunk prefetch)."""
                        pend = []
                        nkt = kpt if null_merged else kpt + 1
                        for kt in range(nkt):
                            rows = krows(kt) if kt < kpt else 1
                            lhs = (
                                kT[:, h_, kt * 128 : kt * 128 + rows]
                                if kt < kpt
                                else kT[:, h_, KP : KP + 1]
                            )
                            pss = ps_sim.tile([rows, QW], F32, tag="sim")
                            for j in range(QW // 512):
                                nc.tensor.matmul(
                                    pss[:, j * 512 : (j + 1) * 512],
                                    lhs,
                                    qT[:, h_, qc_ * QW + j * 512 : qc_ * QW + (j + 1) * 512],
                                    start=True, stop=True,
                                )
                            if kt == 2 and pending_flush is not None:
                                pending_flush()
                                pending_flush = None
                            if (
                                not defer_avs
                                and pending_flush is None
                                and len(pend) >= 2
                            ):
                                emit_av2(pso_, h_, qc_, *pend.pop(0))
                            pt = pt_p.tile([rows, QW], BF16, tag="pt")
                            nc.scalar.activation(
                                pt[:], pss[:], AF.Exp, bias=mb[0:rows, kt : kt + 1]
                            )
                            pend.append((pt, kt))
                        if pending_flush is not None:  # tiny kpt: kt==2 never hit
                            pending_flush()
                        return pend

                    pending = None
                    for h in range(H_PER):
                        if pre and pre[0][0] == h:
                            # this head's sims/exps were prefetched before the
                            # previous chunk's out-proj; drain its AVs now
                            _, pend = pre.pop(0)
                            pso = ps_out.tile([DH + 1, QW], F32, tag="pso")
                            for i, pv in enumerate(pend):
                                if i == 2 and pending is not None:
                                    emit_norm(*pending)
                                    pending = None
                                emit_av2(pso, h, qc, *pv)
                            if pending is not None:  # tiny kpt
                                emit_norm(*pending)
                                pending = None
                        else:
                            pso = ps_out.tile([DH + 1, QW], F32, tag="pso")
                            if pending is not None:
                                pn = pending

                                def _flush(pn=pn):
                                    emit_norm(*pn)

                                pending = None
                            else:
                                _flush = None
                            pend = emit_head_sims(qc, h, pso, _flush)
                            for pv in pend:
                                emit_av2(pso, h, qc, *pv)
                        pending = (h, pso)
                    emit_norm(*pending)
                    pending = None
                    if qc + 1 < N // QW:
                        # prefetch next chunk's first two heads (sims + exps
                        # only, AVs deferred) so Act stays saturated under
                        # this chunk's out-projection
                        pre = [
                            (h_, emit_head_sims(qc + 1, h_, None, defer_avs=True))
                            for h_ in range(2)
                        ]
                    # out projection + chunked ReduceScatter + final LN
                    last_qc = qc == N // QW - 1
                    for st in range(QW // 128):
                        part_s = part_p.tile([128, DIM], BF16, tag="part_s")
                        for ch in range(2):
                            # on the last chunk the sim pool is idle: alternate
                            # pools for 4 out-proj accumulators in flight
                            pool = ps_sim if (last_qc and st % 2) else ps_d
                            psp = pool.tile([128, 512], F32, tag="psp" if pool is ps_d else "sim")
                            for h in range(H_PER):
                                nc.tensor.matmul(
                                    psp[:],
                                    outT[:, h, st * 128 : (st + 1) * 128],
                                    wout[:, h, ch * 512 : (ch + 1) * 512],
                                    start=(h == 0), stop=(h == H_PER - 1),
                                )
                            nc.vector.tensor_copy(
                                part_s[:, ch * 512 : (ch + 1) * 512], psp[:]
                            )
                        row0 = qc * QW + st * 128
                        nc.scalar.dma_start(
                            partial[row0 : row0 + 128, :], part_s[:]
                        )
                        if st == QW // 128 - 1:
                            nc.gpsimd.collective_compute(
                                "ReduceScatter",
                                mybir.AluOpType.add,
                                replica_groups=[[0, 1, 2, 3], [4, 5, 6, 7]],
                                ins=[partial[qc * QW : (qc + 1) * QW, :].opt()],
                                outs=[
                                    rs_out[
                                        qc * (QW // 4) : (qc + 1) * (QW // 4), :
                                    ].opt()
                                ],
                            )
                    if qc > 0:
                        # deferred: previous chunk's final LNs. The huge
                        # wait-until stamp places them at the very end of
                        # each engine queue, so the RS-gated stats can never
                        # head-of-line block normalize/out-proj/exp work.
                        # Real timing is still semaphore-gated by the RS.
                        with tc.tile_wait_until(10 + qc):
                            emit_final_ln(2 * qc - 2)
                            emit_final_ln(2 * qc - 1)
                # last qc's final LNs
                with tc.tile_wait_until(20):
                    emit_final_ln(2 * (N // QW) - 2)
                    emit_final_ln(2 * (N // QW) - 1)

    split_multi_waits(nc)
    return nc


def _prep_inputs(x, context, mask, g_norm, null_kv, Wq, Wkv, Wout, g_out):
    """Host-side sharding + mask compaction + bf16 casts."""
    from ml_dtypes import bfloat16

    x = np.asarray(x, dtype=np.float32)
    context = np.asarray(context, dtype=np.float32)
    mask = np.asarray(mask).astype(bool)
    g_norm = np.asarray(g_norm, dtype=np.float32)
    null_kv = np.asarray(null_kv, dtype=np.float32)
    Wq = np.asarray(Wq, dtype=np.float32)
    Wkv = np.asarray(Wkv, dtype=np.float32)
    Wout = np.asarray(Wout, dtype=np.float32)
    g_out = np.asarray(g_out, dtype=np.float32)

    cnts = [int(mask[b].sum()) for b in range(B)]
    cmax = max(max(cnts), 1)
    kpt = (cmax + 127) // 128
    rlast = cmax - 128 * (kpt - 1)
    KP = kpt * 128

    Wq_g = (g_norm[:, None] * Wq).astype(bfloat16)  # fold g_norm into Wq
    Wk_b = Wkv[:, :INNER].astype(bfloat16)
    Wv_b = Wkv[:, INNER:].astype(bfloat16)
    Wout_b = Wout.astype(bfloat16)
    nk = np.ascontiguousarray(null_kv[0].reshape(DH, 1)).astype(bfloat16)
    nvr = (
        np.concatenate([null_kv[1], [1.0]]).reshape(1, DH + 1).astype(bfloat16)
    )

    def _ptn(w):  # [DIM, HS] -> [128, KT*HS]: partition-major tiled layout
        return np.ascontiguousarray(
            w.reshape(KT, 128, HS).transpose(1, 0, 2).reshape(128, KT * HS)
        )

    def _phn(w):  # [HS, DIM] -> [DH, H_PER*DIM]
        return np.ascontiguousarray(
            w.reshape(H_PER, DH, DIM).transpose(1, 0, 2).reshape(DH, H_PER * DIM)
        )

    ctx_c = []
    mbs = []
    for b in range(B):
        cc = np.zeros((KP, DIM), np.float32)
        cc[: cnts[b]] = context[b][mask[b]]
        ctx_c.append(cc.astype(bfloat16))
        bias = np.where(np.arange(KP) < cnts[b], 0.0, NEG_BIG).astype(np.float32)
        if cmax < KP:
            bias[cmax] = 0.0  # merged null key slot: always attended
        mb = np.zeros((128, kpt + 1), np.float32)
        mb[:, :kpt] = bias.reshape(kpt, 128).T
        mbs.append(mb)

    in_maps = []
    for c in range(8):
        b, hg = c // 4, c % 4
        hs = hg * HS
        in_maps.append(
            {
                "x": np.ascontiguousarray(x[b]).astype(bfloat16),
                "ctx": ctx_c[b],
                "maskbias": mbs[b],
                "nk": nk,
                "nvr": nvr,
                "wq": _ptn(Wq_g[:, hs : hs + HS]),
                "wk": _ptn(Wk_b[:, hs : hs + HS]),
                "wv": _ptn(Wv_b[:, hs : hs + HS]),
                "wout": _phn(Wout_b[hs : hs + HS, :]),
                "gout": g_out,
            }
        )
    return in_maps, kpt, rlast


def _get_program(kpt, rlast):
    if (kpt, rlast) not in _cache:
        _cache[(kpt, rlast)] = build_program(kpt, rlast)
    return _cache[(kpt, rlast)]


def kernel(x, context, mask, g_norm, null_kv, Wq, Wkv, Wout, g_out, _trace=False):
    in_maps, kpt, rlast = _prep_inputs(
        x, context, mask, g_norm, null_kv, Wq, Wkv, Wout, g_out
    )
    nc = _get_program(kpt, rlast)
    res = run_bass_kernel_spmd(nc, in_maps, list(range(8)), trace=_trace)
    out = np.empty((B, N, DIM), np.float32)
    nqc = N // QW
    rows = QW // 4
    for c in range(8):
        b, r = c // 4, c % 4
        yc = res.results[c]["y"]
        for j in range(nqc):
            out[b, j * QW + r * rows : j * QW + (r + 1) * rows, :] = yc[
                j * rows : (j + 1) * rows
            ]
    if _trace:
        return out, res
    return out
